# revision 11
# baseline (speedup 1.0000x reference)
"""Trainium2 Bass kernel for nn_DeChunkLayer (ragged EMA de-chunk).

Math (per batch row b):
    p[l]   = clip(boundary_prob[b, l, 1], EPS, 1-EPS)
    nb[l]  = cumsum_l(boundary_mask[b])          (>= 1 since l=0 is a boundary)
    h(k)   = (1-p_s[k]) h(k-1) + p_s[k] x[k]     (EMA over chunk index k)
    out[l] = h(nb[l]-1)

Collapses to ONE first-order scan in l-space:
    out[l] = a[l]*out[l-1] + bn[l]
    a[l]   = 1 - pm[l],  pm[l] = boundary_mask[l]*p[l]
    bn[l]  = pm[l] * x[nb[l]-1]

Instead of a serial DVE scan, each 128-long l-tile solves the scan in
closed form on the PE (transfer-matrix form):
    out[l] = sum_{l'<=l} T[l,l'] bn[l'] + A[l] h_in
    T[l,l'] = exp(S[l]-S[l']),  S = within-tile inclusive cumsum of ln(a)
    A[l]   = exp(S[l])
T^T is built by ONE scalar-engine Exp over a precomputed
(S-broadcast + lower-triangular -1e9 mask) tile with per-partition bias
-S[l']; masked entries exp(-1e9) = 0, and S decreasing within a tile
means live entries never overflow. Then per tile two bf16 matmuls:
    H = TT^T @ bn  (+)  A_row^T @ h_in     -> PSUM [128 l, 512 d]
which is already l-major: no transposes anywhere, and only a [1,512]
carry copy is serial between tiles.

The gather xg[l] = x[nb[l]-1] uses indirect DMA with a bounds-check
sentinel: non-boundary positions (75%) get idx=20000 > bounds 8191 and
are SKIPPED (no transfer) - gather HBM traffic drops 16 MiB -> ~4.2 MiB.
Skipped rows are garbage (zeros in CoreSim, stale SBUF on HW); they are
multiplied by pm=0 in bn. xg pool buffers are memset once so first-use
stale data is finite (0 * NaN would poison the matmul).

kernel(**inputs) takes FULL inputs, shards over 8 cores (4 batch rows x
2 D-halves), returns FULL (4, 8192, 1024) f32 output.
"""

import os
import sys

import numpy as np

sys.path.insert(0, "/opt/trn_rl_repo")

B, L, D = 4, 8192, 1024
NCORES = 8
DSH = D // 2          # 512 channels per core
NLT = L // 128        # 64 l-tiles of 128
NCH = L // 512        # 16 chunks of 512
EPS = 1e-4

_prog = None  # cached compiled Bass program


def _build_program(reps=1):
    import concourse.bass as bass
    import concourse.mybir as mybir
    from concourse import bacc
    from concourse.bass import IndirectOffsetOnAxis
    from concourse.masks import make_identity, make_upper_triangular
    from concourse.tile import TileContext

    f32 = mybir.dt.float32
    bf16 = mybir.dt.bfloat16
    i32 = mybir.dt.int32
    u8 = mybir.dt.uint8
    Op = mybir.AluOpType
    Act = mybir.ActivationFunctionType

    nc = bacc.Bacc("TRN2", target_bir_lowering=False, debug=False,
                   num_devices=NCORES)

    x = nc.declare_dram_parameter("x", [L, DSH], f32, isOutput=False)
    bp = nc.declare_dram_parameter("bp", [64, 256], f32, isOutput=False)
    bm = nc.declare_dram_parameter("bm", [64, 128], u8, isOutput=False)
    out = nc.declare_dram_parameter("out", [L, DSH], f32, isOutput=True)

    with TileContext(nc) as tc:
        with (
            tc.tile_pool(name="const", bufs=1) as cpool,
            tc.tile_pool(name="prep", bufs=1) as ppool,
        ):
            # ---- constants ----
            ident = cpool.tile([128, 128], f32, tag="ident")
            make_identity(nc, ident[:])
            ut1 = cpool.tile([128, 128], f32, tag="ut1")
            make_upper_triangular(nc, ut1[:], 1.0, diag=True)
            ones1 = cpool.tile([1, 128], f32, tag="ones1")
            nc.vector.memset(ones1[:], 1.0)
            ones_col = cpool.tile([128, 1], f32, tag="ones_col")
            nc.vector.memset(ones_col[:], 1.0)
            zeros1 = cpool.tile([1, 64], f32, tag="zeros1")
            nc.vector.memset(zeros1[:], 0.0)
            # Additive mask for the ROTATED transfer matrix. Output row i of
            # a tile holds l = (i-1) mod 128 (so the carry row l=127 lands on
            # partition 0, which the scalar engine may legally read).
            # Valid (mask 0) iff l >= l' iff (i > l' or i == 0); else -1e9.
            ut0 = cpool.tile([128, 128], f32, tag="ut0")
            make_upper_triangular(nc, ut0[:], 1.0, diag=False)
            nc.vector.memset(ut0[:][:, 0:1], 1.0)
            ltneg = cpool.tile([128, 128], f32, tag="ltneg")
            nc.vector.tensor_scalar(
                out=ltneg[:], in0=ut0[:],
                scalar1=1.0, scalar2=1e9, op0=Op.subtract, op1=Op.mult)

            # ---- scalar preprocessing ----
            # row-major [64,128]: element [p, f] = l = 128*p + f
            bm_u8 = ppool.tile([64, 128], u8, tag="bm_u8")
            nc.sync.dma_start(out=bm_u8[:], in_=bm[:])
            bmf = ppool.tile([64, 128], f32, tag="bmf")
            nc.vector.tensor_copy(bmf[:], bm_u8[:])

            bp_rm = ppool.tile([64, 256], f32, tag="bp_rm")
            nc.sync.dma_start(out=bp_rm[:], in_=bp[:])
            p_rm = ppool.tile([64, 128], f32, tag="p_rm")
            nc.vector.tensor_scalar(
                out=p_rm[:], in0=bp_rm[:][:, 1::2],
                scalar1=EPS, scalar2=1.0 - EPS, op0=Op.max, op1=Op.min)
            pm_rm = ppool.tile([64, 128], f32, tag="pm_rm")
            nc.vector.tensor_tensor(
                out=pm_rm[:], in0=p_rm[:], in1=bmf[:], op=Op.mult)
            arow_rm = ppool.tile([64, 128], f32, tag="arow_rm")
            nc.vector.tensor_scalar(
                out=arow_rm[:], in0=pm_rm[:],
                scalar1=-1.0, scalar2=1.0, op0=Op.mult, op1=Op.add)
            la_rm = ppool.tile([64, 128], f32, tag="la_rm")
            nc.scalar.activation(out=la_rm[:], in_=arow_rm[:], func=Act.Ln)

            bm_cm = ppool.tile([128, 64], f32, tag="bm_cm")
            pm_cm = ppool.tile([128, 64], f32, tag="pm_cm")
            la_cm = ppool.tile([128, 64], f32, tag="la_cm")
            colsum = ppool.tile([1, 64], f32, tag="colsum")
            csum = ppool.tile([1, 64], f32, tag="csum")
            excl = ppool.tile([1, 64], f32, tag="excl")
            idxf = ppool.tile([128, 64], f32, tag="idxf")
            sent = ppool.tile([128, 64], f32, tag="sent")
            idxm_f = ppool.tile([128, 64], f32, tag="idxm_f")
            idxm = ppool.tile([128, 64], i32, tag="idxm")
            sneg_cm = ppool.tile([128, 64], f32, tag="sneg_cm")
            s_cm = ppool.tile([128, 64], f32, tag="s_cm")
            s_rm = ppool.tile([64, 128], f32, tag="s_rm")
            a_rm = ppool.tile([64, 128], bf16, tag="a_rm")
            s_flat = ppool.tile([1, L], f32, tag="s_flat")
            a_flat = ppool.tile([1, L], bf16, tag="a_flat")
            sbm = ppool.tile([128, L], f32, tag="sbm")

            with tc.tile_pool(name="pps", bufs=1, space="PSUM") as pps:
                # col-major [128,64]: element [q, g] = l = q + 128*g
                bmT_ps = pps.tile([128, 64], f32, tag="bmT")
                nc.tensor.transpose(out=bmT_ps[:], in_=bmf[:],
                                    identity=ident[:][:64, :64])
                nc.vector.tensor_copy(bm_cm[:], bmT_ps[:])
                pmT_ps = pps.tile([128, 64], f32, tag="pmT")
                nc.tensor.transpose(out=pmT_ps[:], in_=pm_rm[:],
                                    identity=ident[:][:64, :64])
                nc.vector.tensor_copy(pm_cm[:], pmT_ps[:])
                laT_ps = pps.tile([128, 64], f32, tag="laT")
                nc.tensor.transpose(out=laT_ps[:], in_=la_rm[:],
                                    identity=ident[:][:64, :64])
                nc.vector.tensor_copy(la_cm[:], laT_ps[:])

                # nb = within-column inclusive cumsum + per-column offsets
                nb_ps = pps.tile([128, 64], f32, tag="nb")
                nc.tensor.matmul(out=nb_ps[:], lhsT=ut1[:], rhs=bm_cm[:],
                                 start=True, stop=False)
                cs_ps = pps.tile([1, 64], f32, tag="cs")
                nc.tensor.matmul(out=cs_ps[:], lhsT=ones_col[:], rhs=bm_cm[:],
                                 start=True, stop=True)
                nc.vector.tensor_copy(colsum[:], cs_ps[:])
                nc.vector.tensor_tensor_scan(
                    out=csum[:], data0=colsum[:], data1=zeros1[:],
                    initial=0.0, op0=Op.add, op1=Op.add)
                nc.vector.tensor_tensor(
                    out=excl[:], in0=csum[:], in1=colsum[:], op=Op.subtract)
                nc.tensor.matmul(out=nb_ps[:], lhsT=ones1[:], rhs=excl[:],
                                 start=False, stop=True)

                # idx = max(nb-1, 0) col-major; sentinel 20000 off-boundary
                # (> bounds_check 8191 -> gather row skipped)
                nc.vector.tensor_scalar(
                    out=idxf[:], in0=nb_ps[:],
                    scalar1=1.0, scalar2=0.0, op0=Op.subtract, op1=Op.max)
                nc.vector.tensor_scalar(
                    out=sent[:], in0=bm_cm[:],
                    scalar1=-20000.0, scalar2=20000.0,
                    op0=Op.mult, op1=Op.add)
                nc.vector.tensor_tensor(
                    out=idxm_f[:], in0=idxf[:], in1=sent[:], op=Op.add)
                nc.vector.tensor_copy(idxm[:], idxm_f[:])

                # within-tile inclusive cumsum of ln(a): S (col-major)
                s_ps = pps.tile([128, 64], f32, tag="s_ps")
                nc.tensor.matmul(out=s_ps[:], lhsT=ut1[:], rhs=la_cm[:],
                                 start=True, stop=True)
                nc.vector.tensor_scalar(
                    out=sneg_cm[:], in0=s_ps[:], scalar1=-1.0, scalar2=None,
                    op0=Op.mult)
                nc.vector.tensor_copy(s_cm[:], s_ps[:])
                srm_ps = pps.tile([64, 128], f32, tag="srm_ps")
                nc.tensor.transpose(out=srm_ps[:], in_=s_cm[:],
                                    identity=ident[:])
                # A = exp(S) in bf16, row j = l-tile j (carry matmul lhsT)
                nc.scalar.activation(out=a_rm[:], in_=srm_ps[:], func=Act.Exp)
                nc.vector.tensor_copy(s_rm[:], srm_ps[:])

            # Rotated S as [1, 8192] (position 128j+i = S_j[(i-1) % 128]),
            # bounced through DRAM scratch, then broadcast down 128
            # partitions chunk-by-chunk via DMA (stride-0 DRAM source),
            # then add the rotated triangular mask in one DVE pass.
            # a_flat gets the same rotation (carry-matmul lhsT).
            s_dram = nc.dram_tensor("s_scratch", [64, 128], f32)
            nc.sync.dma_start(out=s_dram[:][:, 0:1], in_=s_rm[:][:, 127:128])
            nc.sync.dma_start(out=s_dram[:][:, 1:128], in_=s_rm[:][:, 0:127])
            nc.sync.dma_start(
                out=a_flat[:][0:1, :].rearrange(
                    "o (j i) -> o j i", i=128)[:, :, 0:1],
                in_=a_rm[:][:, 127:128])
            nc.sync.dma_start(
                out=a_flat[:][0:1, :].rearrange(
                    "o (j i) -> o j i", i=128)[:, :, 1:128],
                in_=a_rm[:][:, 0:127])
            sd_flat = s_dram[:].rearrange("j i -> () (j i)")
            for c in range(NCH):
                nc.sync.dma_start(
                    out=sbm[:][:, 512 * c:512 * (c + 1)],
                    in_=sd_flat[0:1, 512 * c:512 * (c + 1)].to_broadcast(
                        (128, 512)))
            nc.vector.tensor_tensor(
                out=sbm[:].rearrange("p (t f) -> p t f", t=NLT),
                in0=sbm[:].rearrange("p (t f) -> p t f", t=NLT),
                in1=ltneg[:][:, None, :].to_broadcast((128, NLT, 128)),
                op=Op.add)

            # ---- main loop over 16 chunks of 512 positions ----
            with (
                tc.tile_pool(name="xg", bufs=4) as xgp,
                tc.tile_pool(name="bn", bufs=8) as bnp,
                tc.tile_pool(name="tt", bufs=8) as ttp,
                tc.tile_pool(name="hp", bufs=6) as hpp,
                tc.tile_pool(name="H", bufs=5, space="PSUM") as Hp,
                tc.tile_pool(name="ost", bufs=3) as ostp,
            ):
                # memset gather buffers once: skipped rows must read
                # finite stale data (0 * NaN = NaN would poison the MACs)
                for i in range(4):
                    t = xgp.tile([128, 2048], f32, tag="xg", name=f"xgi{i}")
                    nc.vector.memset(t[:], 0.0)

                h_prev = hpp.tile([1, 512], bf16, tag="h", name="h_init")
                nc.vector.memset(h_prev[:], 0.0)

                def front(c, rep, h_prev):
                    xg4 = xgp.tile([128, 2048], f32, tag="xg",
                                   name=f"xg_{c}_{rep}")
                    for jj in range(4):
                        nc.gpsimd.indirect_dma_start(
                            out=xg4[:][:, 512 * jj:512 * (jj + 1)],
                            out_offset=None, in_=x[:],
                            in_offset=IndirectOffsetOnAxis(
                                ap=idxm[:][:, 4 * c + jj:4 * c + jj + 1],
                                axis=0),
                            bounds_check=L - 1, oob_is_err=False)

                    ost = ostp.tile([128, 2048], f32, tag="ost",
                                    name=f"ost_{c}_{rep}")
                    for jj in range(4):
                        j = 4 * c + jj
                        tt = ttp.tile([128, 128], bf16, tag="tt",
                                      name=f"tt_{c}_{jj}_{rep}")
                        nc.scalar.activation(
                            out=tt[:], in_=sbm[:][:, 128 * j:128 * (j + 1)],
                            func=Act.Exp, bias=sneg_cm[:][:, j:j + 1],
                            scale=1.0)
                        bn = bnp.tile([128, 512], bf16, tag="bn",
                                      name=f"bn_{c}_{jj}_{rep}")
                        nc.vector.tensor_scalar_mul(
                            bn[:], xg4[:][:, 512 * jj:512 * (jj + 1)],
                            pm_cm[:][:, j:j + 1])
                        H = Hp.tile([128, 512], f32, tag="H",
                                    name=f"H_{c}_{jj}_{rep}")
                        nc.tensor.matmul(out=H[:], lhsT=tt[:], rhs=bn[:],
                                         start=True, stop=False)
                        nc.tensor.matmul(
                            out=H[:],
                            lhsT=a_flat[:][0:1, 128 * j:128 * (j + 1)],
                            rhs=h_prev[:], start=False, stop=True)
                        h_new = hpp.tile([1, 512], bf16, tag="h",
                                         name=f"h_{c}_{jj}_{rep}")
                        nc.scalar.copy(out=h_new[:], in_=H[:][0:1, :])
                        nc.scalar.copy(
                            out=ost[:][:, 512 * jj:512 * (jj + 1)], in_=H[:])
                        h_prev = h_new
                    return ost, h_prev

                def back(c, rep, ost):
                    # undo the row rotation: ost partition i holds
                    # l = 512c + 128b + (i-1) mod 128
                    o = out[:][512 * c:512 * (c + 1), :].rearrange(
                        "(b a) d -> a b d", a=128)
                    nc.sync.dma_start(
                        out=o[0:127, :, :],
                        in_=ost[:][1:128, :].rearrange("a (b d) -> a b d", b=4))
                    nc.sync.dma_start(
                        out=o[127:128, :, :],
                        in_=ost[:][0:1, :].rearrange("a (b d) -> a b d", b=4))

                # software-pipelined emission: front of chunk c+1 before
                # the output DMA of chunk c.
                for rep in range(reps):
                    prev = None
                    for c in range(NCH):
                        ost, h_prev = front(c, rep, h_prev)
                        if prev is not None:
                            back(prev[0], rep, prev[1])
                        prev = (c, ost)
                    back(prev[0], rep, prev[1])

    nc.compile()
    return nc


def _install_profile_hook():
    """Provide antenv.axon_hooks (missing in this image) so
    run_bass_kernel_spmd(trace=True) can capture NTFF profiles via
    /opt/axon/libaxon_pjrt.so."""
    import sys as _sys
    import types
    import contextlib
    import ctypes

    if "antenv.axon_hooks" in _sys.modules:
        return
    try:
        lib = ctypes.CDLL("/opt/axon/libaxon_pjrt.so")
        if not hasattr(lib, "axon_start_nrt_profile"):
            return
    except OSError:
        return
    lib.axon_start_nrt_profile.argtypes = [
        ctypes.POINTER(ctypes.c_int64), ctypes.c_size_t]
    lib.axon_start_nrt_profile.restype = ctypes.c_int64
    lib.axon_stop_nrt_profile.argtypes = [ctypes.c_char_p]
    lib.axon_stop_nrt_profile.restype = ctypes.c_int64

    @contextlib.contextmanager
    def _hook(output_dir, device_ids):
        import jax
        jax.devices()
        if device_ids:
            ids = (ctypes.c_int64 * len(device_ids))(*device_ids)
            rc = lib.axon_start_nrt_profile(ids, len(device_ids))
        else:
            rc = lib.axon_start_nrt_profile(None, 0)
        if rc != 0:
            raise RuntimeError(f"axon_start_nrt_profile rc={rc}")
        try:
            yield
        finally:
            n = lib.axon_stop_nrt_profile(str(output_dir).encode())
            print(f"profile: {n} file(s) written to {output_dir}",
                  file=sys.stderr)

    m = types.ModuleType("antenv.axon_hooks")
    m.get_axon_ntff_profile_hook = lambda: _hook
    m.set_axon_ntff_profile_hook = lambda h: None
    _sys.modules["antenv.axon_hooks"] = m


def _get_program():
    global _prog
    if _prog is None:
        _prog = _build_program()
    return _prog


def run(inputs, trace=False):
    """Returns (full_output, exec_time_ns or None)."""
    from concourse.bass_utils import run_bass_kernel_spmd

    hidden_states = np.asarray(inputs["hidden_states"], dtype=np.float32)
    boundary_mask = np.asarray(inputs["boundary_mask"])
    boundary_prob = np.asarray(inputs["boundary_prob"], dtype=np.float32)

    nc = _get_program()
    in_maps = []
    for c in range(NCORES):
        b, h = divmod(c, 2)
        in_maps.append({
            "x": np.ascontiguousarray(hidden_states[b, :, h * DSH:(h + 1) * DSH]),
            "bp": np.ascontiguousarray(boundary_prob[b].reshape(64, 256)),
            "bm": np.ascontiguousarray(
                boundary_mask[b].astype(np.uint8).reshape(64, 128)),
        })
    if trace:
        _install_profile_hook()
    res = run_bass_kernel_spmd(nc, in_maps, list(range(NCORES)), trace=trace)
    outs = res.results
    full = np.empty((B, L, D), np.float32)
    for c in range(NCORES):
        b, h = divmod(c, 2)
        full[b, :, h * DSH:(h + 1) * DSH] = outs[c]["out"]
    return full, res.exec_time_ns


def kernel(**inputs) -> np.ndarray:
    out, _ = run(inputs, trace=False)
    return out
